# revision 51
# baseline (speedup 1.0000x reference)
"""Trainium2 Bass kernel: 6-layer decoder (masked self-attn + cross-attn + FFN).

Sharding (8 cores): 4 batch pairs x 2-way sequence-parallel.
Core r: batch r//2, half g=r%2. Global 512-token chunks: g=0 owns [c0,c3],
g=1 owns [c1,c2] (zigzag for causal load balance). Activations are stored
core-locally as [my 1024 tokens | peer 1024 tokens]; causal structure is
identical across cores (union schedule), with per-core differences expressed
purely as data (exp-bias columns and diagonal mask constants).
One AllGather per layer (within each pair) exchanges the layer output.

Layout: feature-major ("T") everywhere: actT[feat, tok]. Matmuls compute
outT = W(lhsT).T @ actT(rhs) with float32r; scores/AV run in bf16.
"""

import os

import numpy as np
import ml_dtypes

import concourse.bass as bass
import concourse.mybir as mybir
import concourse.tile as tile
from concourse import bacc
from concourse.bass import ts
from concourse.bass_utils import run_bass_kernel_spmd

L, B, S, D, H, DK, F = 6, 4, 2048, 512, 8, 64, 2048
P = 128
TCH = 512                 # token chunk = matmul free dim
HALF = S // 2             # tokens owned per core
KC = D // P               # 4 partition chunks of d_model
NFT = F // P              # 16 feature tiles of FFN hidden
NKT = S // P              # 16 k-tiles over full sequence
AVW = DK + 1              # V columns per head + ones column (softmax sum)
EPS = 1e-5
SCALE = 1.0 / float(np.sqrt(DK))
NEG = -1e9

f32 = mybir.dt.float32
f32r = mybir.dt.float32r
bf16 = mybir.dt.bfloat16
fp8 = mybir.dt.float8e4
PM = mybir.MatmulPerfMode
AF = mybir.ActivationFunctionType
ALU = mybir.AluOpType

NLAYERS = int(os.environ.get("KERNEL_NLAYERS", str(L)))
OPT_ACTSET = bool(int(os.environ.get("KOPT_ACTSET", "1")))
OPT_GMASK = bool(int(os.environ.get("KOPT_GMASK", "1")))
WAIT_MS = float(os.environ.get("KOPT_WAIT_MS", "0.5"))
RG = [[0, 1], [2, 3], [4, 5], [6, 7]]

# Union causal schedule (identical on every core). Local k-tile order:
# 0-3 = my chunk j0, 4-7 = my chunk j1, 8-11 = peer j0, 12-15 = peer j1.
SA_KT = {0: [0, 1, 2, 3, 8, 9, 10, 11], 1: list(range(16))}
CA_KT = {0: list(range(16)), 1: list(range(16))}


def _diag_shape(j, kt):
    if j == 0 and kt < 4:
        return kt
    if j == 1 and 4 <= kt < 8:
        return kt - 4
    return None


def _bias_col(j, kt):
    if kt >= 8:
        return (kt - 8) if j == 0 else 4 + (kt - 8)
    return None


def _r(ap):
    return ap.bitcast(f32r)


def _single_act_set():
    # Force every ACT function onto natural_log_exp_and_others (it contains
    # Exp, Ln, Identity and Relu) so the compiled kernel has exactly one
    # ACT_TABLE_LOAD instead of thrashing between per-function sets.
    real = bacc.get_activation_tables

    def patched(arch):
        tabs = real(arch)
        return {name: (fns if name == "natural_log_exp_and_others" else set())
                for name, fns in tabs.items()}

    bacc.get_activation_tables = patched


if OPT_ACTSET:
    _single_act_set()


def build(ln_affine: bool, v_bias: bool):
    nc = bacc.Bacc(None, target_bir_lowering=False, num_devices=8)

    xT = nc.declare_dram_parameter("xT", [P, KC, S], bf16, isOutput=False)
    encT = nc.declare_dram_parameter("encT", [P, KC, S], bf16, isOutput=False)
    w_in = {}
    for pre in ("sa", "ca"):
        for nm in ("wq", "wk", "wv"):
            w_in[f"{pre}_{nm}"] = nc.declare_dram_parameter(f"{pre}_{nm}", [L, D, D], bf16, isOutput=False)
        w_in[f"{pre}_wo"] = nc.declare_dram_parameter(f"{pre}_wo", [L, D, D], bf16, isOutput=False)
        for nm in ("bq", "bk", "bv", "bo"):
            w_in[f"{pre}_{nm}"] = nc.declare_dram_parameter(f"{pre}_{nm}", [L, D], f32, isOutput=False)
    w_in["ff_w1"] = nc.declare_dram_parameter("ff_w1", [L, D, F], bf16, isOutput=False)
    w_in["ff_b1"] = nc.declare_dram_parameter("ff_b1", [L, F], f32, isOutput=False)
    w_in["ff_w2b"] = nc.declare_dram_parameter("ff_w2b", [L, F, D], bf16, isOutput=False)
    w_in["ff_b2"] = nc.declare_dram_parameter("ff_b2", [L, D], f32, isOutput=False)
    if ln_affine:
        for i in (1, 2, 3):
            w_in[f"ln{i}_g"] = nc.declare_dram_parameter(f"ln{i}_g", [L, D], f32, isOutput=False)
            w_in[f"ln{i}_b"] = nc.declare_dram_parameter(f"ln{i}_b", [L, D], f32, isOutput=False)
    ones_in = nc.declare_dram_parameter("ones", [P, P], f32r, isOutput=False)
    identb_in = nc.declare_dram_parameter("identb", [P, P], bf16, isOutput=False)
    dmask_in = nc.declare_dram_parameter("dmask", [P, 4, TCH], bf16, isOutput=False)
    pbias_in = nc.declare_dram_parameter("pbias", [P, 12], f32, isOutput=False)
    out_p = nc.declare_dram_parameter("out", [P, KC, HALF], f32, isOutput=True)

    with tile.TileContext(nc, num_cores=8) as tc:
        import contextlib

        gctx = contextlib.ExitStack()
        with gctx:
            persist = gctx.enter_context(tc.tile_pool(name="persist", bufs=1))
            psA = gctx.enter_context(tc.tile_pool(name="psA", bufs=2, space="PSUM"))
            psS = gctx.enter_context(tc.tile_pool(name="psS", bufs=2, space="PSUM"))
            psO = gctx.enter_context(tc.tile_pool(name="psO", bufs=2, space="PSUM"))
            dramp = gctx.enter_context(tc.tile_pool(name="dramp", bufs=2, space="DRAM"))
            # SA qkv weights double-buffered across layers: prefetched during
            # layer l-1's FFN so kv/q projections can run under the AllGather
            wsa = gctx.enter_context(tc.tile_pool(name="wsa", bufs=2))

            # own and peer halves as separate tiles so the inter-core gather
            # (write to hPeerT) doesn't serialize against own-half readers
            hOwnT = persist.tile([P, KC, HALF], bf16, name="hOwnT")
            hPeerT = persist.tile([P, KC, HALF], bf16, name="hPeerT")
            outT = persist.tile([P, KC, HALF], f32r, name="outT")
            ones_sb = persist.tile([P, P], f32r, name="ones_sb")
            identb_sb = persist.tile([P, P], bf16, name="identb_sb")
            dmask_sb = persist.tile([P, 4, TCH], bf16, name="dmask_sb")
            pbias_sb = persist.tile([P, 12], f32, name="pbias_sb")
            zero_sb = persist.tile([P, 1], f32, name="zero_sb")
            eps_sb = persist.tile([P, 1], f32, name="eps_sb")
            nc.vector.memset(zero_sb, 0.0)
            nc.vector.memset(eps_sb, EPS)

            for kc in range(KC):
                nc.sync.dma_start(out=hOwnT[:, kc, :], in_=xT[:, kc, 0:HALF])
                nc.sync.dma_start(out=hPeerT[:, kc, :], in_=xT[:, kc, HALF:S])

            def h_src(t):  # token chunk t of the local [own|peer] layout
                src = hOwnT if t < 2 else hPeerT
                tt = t % 2
                return src[:, :, tt * TCH:(tt + 1) * TCH]
            nc.sync.dma_start(out=ones_sb, in_=ones_in[:, :])
            nc.sync.dma_start(out=identb_sb, in_=identb_in[:, :])
            nc.sync.dma_start(out=dmask_sb, in_=dmask_in[:, :, :])
            nc.sync.dma_start(out=pbias_sb, in_=pbias_in[:, :])

            pid = nc.sync.partition_id()
            peer = (pid + 1) % 2
            pid_act = nc.scalar.partition_id()
            peer_act = (pid_act + 1) % 2

            def load_w(pool, dram_t, l, cols, dt=bf16, tag="w", bufs=3):
                n = dram_t.shape[1] // P
                l = l % L
                t = pool.tile([P, n, cols], dt, tag=tag, bufs=bufs, name=tag)
                for kc in range(n):
                    nc.sync.dma_start(out=t[:, kc, :], in_=dram_t[l, kc * P:(kc + 1) * P, :])
                return t

            def load_b(pool, dram_t, l, tag):
                n = dram_t.shape[1] // P
                l = l % L
                t = pool.tile([P, n], f32, tag=tag, bufs=2, name=tag)
                nc.sync.dma_start(out=t, in_=dram_t[l].rearrange("(c p) -> p c", p=P))
                return t

            def copy_ps(dst, src_ps, bias_ap, eng):
                # psum -> sbuf copy with fused per-partition bias add (DVE;
                # ACT is the busier engine)
                nc.vector.tensor_scalar(dst, src_ps, bias_ap, None, ALU.add)

            def load_sa_weights(l):
                saw = {
                    "wq": load_w(wsa, w_in["sa_wq"], l, D, tag="swq", bufs=2),
                    "wk": load_w(wsa, w_in["sa_wk"], l, D, tag="swk", bufs=2),
                    "wv": load_w(wsa, w_in["sa_wv"], l, D, tag="swv", bufs=2),
                    "bq": load_b(wsa, w_in["sa_bq"], l, "sbq"),
                    "bk": load_b(wsa, w_in["sa_bk"], l, "sbk"),
                    "bv": None,
                }
                if v_bias:
                    bv_t = wsa.tile([1, D], f32, tag="sbv", bufs=2, name="sbv")
                    nc.sync.dma_start(out=bv_t, in_=w_in["sa_bv"][l % L:l % L + 1, :])
                    saw["bv"] = bv_t
                return saw

            def kv_proj(lp, src_getter, wk_sb, bk_sb, wv_sb, bv_sb, kT_t, vaug_t,
                        ts_list=(0, 1, 2, 3), memset_first=True):
                # K^T feature-major (bf16) over full 2048 tokens + V token-major
                # into per-head augmented layout [P, NKT, H*AVW] (bf16).
                if memset_first:
                    # only the per-head ones-columns (col DK of each AVW group)
                    ones_cols = vaug_t[:, :, :].rearrange(
                        "p n (h w) -> p n h w", h=H)[:, :, :, DK:AVW]
                    nc.vector.memset(ones_cols, 1.0)
                for t in ts_list:
                    src = src_getter(t)  # [P, KC, TCH]
                    for ft in range(KC):
                        k_ps = psA.tile([P, TCH], f32, tag="acc", name="k_ps")
                        for kc in range(KC):
                            nc.tensor.matmul(k_ps, wk_sb[:, kc, ft * P:(ft + 1) * P],
                                             src[:, kc, :], start=(kc == 0), stop=(kc == KC - 1))
                        copy_ps(kT_t[:, ft, t * TCH:(t + 1) * TCH], k_ps, bk_sb[:, ft:ft + 1], (t + ft) % 2)
                    for tl in range(4):
                        tt = t * 4 + tl
                        v_ps = psA.tile([P, D], f32, tag="acc", name="v_ps")
                        nmm = KC + (1 if v_bias else 0)
                        for kc in range(KC):
                            nc.tensor.matmul(v_ps, src[:, kc, tl * P:(tl + 1) * P],
                                             wv_sb[:, kc, :], start=(kc == 0),
                                             stop=(kc == nmm - 1))
                        if v_bias:
                            nc.tensor.matmul(v_ps, ones_sb[0:1, :], bv_sb, start=False, stop=True)
                        # one strided copy: v_ps [P, (h d)] -> vaug cols 0:DK per head
                        nc.vector.tensor_copy(
                            out=vaug_t[:, tt, :].rearrange(
                                "p (h w) -> p h w", h=H)[:, :, 0:DK],
                            in_=v_ps[:, :].rearrange("p (h w) -> p h w", h=H))

            def q_proj(lp, srcT, wq_sb, bq_sb, qT_t):
                for t in range(2):
                    for ft in range(KC):
                        q_ps = psA.tile([P, TCH], f32, tag="acc", name="q_ps")
                        for kc in range(KC):
                            nc.tensor.matmul(q_ps, wq_sb[:, kc, ft * P:(ft + 1) * P],
                                             srcT[:, kc, t * TCH:(t + 1) * TCH],
                                             start=(kc == 0), stop=(kc == KC - 1))
                        copy_ps(qT_t[:, ft, t * TCH:(t + 1) * TCH], q_ps, bq_sb[:, ft:ft + 1], (t + ft) % 2)

            # SA pair schedule per j: (kt0, dmask col or None) own / (kt0,
            # pbias col) peer.  Pairs of adjacent k-tiles share bias/mask cols.
            SA_OWN = {0: [(0, 0), (2, 2)],
                      1: [(0, None), (2, None), (4, 0), (6, 2)]}
            SA_PEER = {0: [(8, 0), (10, 0)],
                       1: [(8, 4), (10, 4), (12, 8), (14, 8)]}

            def pair_block(pp, kT_t, vaug_t, qT_t, h, j, kt0, dsh, bias, o_ps,
                           av_start, av_stop):
                s_ps = psS.tile([P, 2, TCH], f32, tag="sc", name="s_ps")
                for q in range(2):
                    kt = kt0 + q
                    nc.tensor.matmul(
                        s_ps[:, q, :],
                        kT_t[(h % 2) * DK:(h % 2) * DK + DK, h // 2, kt * P:(kt + 1) * P],
                        qT_t[(h % 2) * DK:(h % 2) * DK + DK, h // 2, j * TCH:(j + 1) * TCH],
                        start=True, stop=True)
                pt = pp.tile([P, 2, TCH], bf16, tag="pt", bufs=3, name="pt")
                nc.scalar.activation(pt, s_ps, AF.Exp, bias=bias, scale=SCALE)
                if dsh is not None:
                    # DVE, not Pool: the Pool queue carries the collective
                    nc.vector.tensor_mul(pt, pt, dmask_sb[:, dsh:dsh + 2, :])
                for q in range(2):
                    kt = kt0 + q
                    nc.tensor.matmul(o_ps, vaug_t[:, kt, h * AVW:(h + 1) * AVW],
                                     pt[:, q, :], start=(av_start and q == 0),
                                     stop=(av_stop and q == 1))

            def attn_finish(lp, oT_t, h, j, o_ps):
                # 1/sum on DVE straight from the PSUM ones-row, then a PE
                # broadcast matmul replicates it across DK partitions
                srow = lp.tile([AVW, TCH], f32r, tag="srow", bufs=2, name="srow")
                with nc.allow_low_precision(reason="f32r-typed f32 data"):
                    nc.vector.reciprocal(out=srow[DK:AVW, :], in_=o_ps[DK:AVW, :])
                r_ps = psA.tile([DK, TCH], f32, tag="acc", name="r_ps")
                nc.tensor.matmul(r_ps, ones_sb[DK:DK + 1, 0:DK], srow[DK:AVW, :],
                                 start=True, stop=True)
                # DVE may read only one PSUM operand: stage rb in SBUF
                rb = lp.tile([DK, TCH], f32, tag="rb", bufs=2, name="rb")
                nc.vector.tensor_copy(out=rb, in_=r_ps)
                nc.vector.tensor_mul(
                    oT_t[(h % 2) * DK:(h % 2) * DK + DK, h // 2, j * TCH:(j + 1) * TCH],
                    o_ps[0:DK, :], rb)

            def attention_ca(lp, pp, kT_t, vaug_t, qT_t, oT_t):
                zb = zero_sb[:, 0:1]
                for h in range(H):
                    for j in range(2):
                        o_ps = psO.tile([AVW, TCH], f32, tag="oacc", name="o_ps")
                        for i in range(NKT // 2):
                            pair_block(pp, kT_t, vaug_t, qT_t, h, j, 2 * i, None,
                                       zb, o_ps, i == 0, i == NKT // 2 - 1)
                        attn_finish(lp, oT_t, h, j, o_ps)

            def attention_sa_own(pp, kT_t, vaug_t, qT_t, oPart):
                # pass 1: own-token chunks only — no dependence on the gather;
                # partial accumulators (incl. softmax-sum row) spill to SBUF
                zb = zero_sb[:, 0:1]
                for h in range(H):
                    for j in range(2):
                        o_ps = psO.tile([AVW, TCH], f32, tag="oacc", name="o_ps")
                        own = SA_OWN[j]
                        for i, (kt0, dsh) in enumerate(own):
                            pair_block(pp, kT_t, vaug_t, qT_t, h, j, kt0, dsh,
                                       zb, o_ps, i == 0, i == len(own) - 1)
                        nc.vector.tensor_copy(out=oPart[:, 2 * h + j, :], in_=o_ps)

            def attention_sa_peer(lp, pp, kT_t, vaug_t, qT_t, oT_t, oPart, identb):
                # pass 2: re-inject spilled partials, add peer-token chunks
                for h in range(H):
                    for j in range(2):
                        o_ps = psO.tile([AVW, TCH], f32, tag="oacc", name="o_ps")
                        nc.tensor.matmul(o_ps, identb[0:AVW, 0:AVW],
                                         oPart[:, 2 * h + j, :], start=True, stop=False)
                        peer = SA_PEER[j]
                        for i, (kt0, bcol) in enumerate(peer):
                            pair_block(pp, kT_t, vaug_t, qT_t, h, j, kt0, None,
                                       pbias_sb[:, bcol:bcol + 1], o_ps,
                                       False, i == len(peer) - 1)
                        attn_finish(lp, oT_t, h, j, o_ps)

            def out_proj(lp, oT_t, wo_sb, bo_sb, resT, u_t):
                # u = oT @ wo + bo + res   (res fused into the psum evacuation)
                for ft in range(KC):
                    pss = [psA.tile([P, TCH], f32, tag="acc", name="u_ps") for _ in range(2)]
                    for kc in range(KC):
                        for t in range(2):
                            nc.tensor.matmul(pss[t], wo_sb[:, kc, ft * P:(ft + 1) * P],
                                             oT_t[:, kc, t * TCH:(t + 1) * TCH],
                                             start=(kc == 0), stop=(kc == KC - 1))
                    for t in range(2):
                        nc.vector.scalar_tensor_tensor(
                            u_t[:, ft, t * TCH:(t + 1) * TCH], pss[t],
                            bo_sb[:, ft:ft + 1], resT(ft, t), ALU.add, ALU.add)

            def layernorm(lp, rows, u_t, dst, g_sb, b_sb):
                for t in range(2):
                    tsl = slice(t * TCH, (t + 1) * TCH)
                    usq = lp.tile([P, KC, TCH], f32r, tag="usq", bufs=1, name="usq")
                    for kc in range(KC):
                        nc.vector.tensor_mul(usq[:, kc, :], u_t[:, kc, tsl], u_t[:, kc, tsl])
                    m_ps = psS.tile([P, TCH], f32, tag="sc", name="m_ps")
                    for kc in range(KC):
                        nc.tensor.matmul(m_ps, ones_sb, u_t[:, kc, tsl],
                                         start=(kc == 0), stop=(kc == KC - 1))
                    q_ps = psS.tile([P, TCH], f32, tag="sc", name="q_ps")
                    for kc in range(KC):
                        nc.tensor.matmul(q_ps, ones_sb, usq[:, kc, :],
                                         start=(kc == 0), stop=(kc == KC - 1))
                    t_sb = rows.tile([P, TCH], f32, tag="rows", bufs=4, name="t_sb")
                    nc.vector.tensor_scalar(t_sb, m_ps, 1.0 / D, None, ALU.mult)
                    m2 = rows.tile([P, TCH], f32, tag="rows", bufs=4, name="m2")
                    nc.vector.tensor_scalar(m2, q_ps, 1.0 / D, None, ALU.mult)
                    tt2 = rows.tile([P, TCH], f32, tag="rows", bufs=4, name="tt2")
                    nc.vector.tensor_mul(tt2, t_sb, t_sb)
                    nc.vector.tensor_sub(m2, m2, tt2)
                    nc.scalar.activation(m2, m2, AF.Ln, bias=eps_sb[:, 0:1])
                    r_sb = rows.tile([P, TCH], f32, tag="rows", bufs=4, name="r_sb")
                    nc.scalar.activation(r_sb, m2, AF.Exp, scale=-0.5, bias=zero_sb[:, 0:1])
                    c_sb = rows.tile([P, TCH], f32, tag="rows", bufs=4, name="c_sb")
                    nc.vector.tensor_mul(c_sb, t_sb, r_sb)
                    for kc in range(KC):
                        tmp = rows.tile([P, TCH], f32, tag="ltmp", bufs=2, name="ltmp")
                        nc.vector.tensor_sub(tmp, u_t[:, kc, tsl], c_sb)
                        d = dst(kc, t)
                        nc.vector.tensor_mul(d, tmp, r_sb)
                        if ln_affine:
                            nc.vector.tensor_scalar(d, d, g_sb[:, kc:kc + 1], b_sb[:, kc:kc + 1],
                                                    ALU.mult, ALU.add)

            saw = load_sa_weights(0)
            for l in range(NLAYERS):
                lctx = contextlib.ExitStack()
                with lctx:
                    lp = lctx.enter_context(tc.tile_pool(name=f"lay{l}", bufs=1))
                    rows = lctx.enter_context(tc.tile_pool(name=f"rows{l}", bufs=1))
                    x1T = lp.tile([P, KC, HALF], bf16, tag="x1", name="x1T")
                    yT = lp.tile([P, KC, HALF], bf16, tag="y", name="yT")

                    # ---- self-attention ----
                    sctx = contextlib.ExitStack()
                    with sctx:
                        sp = sctx.enter_context(tc.tile_pool(name=f"sa{l}", bufs=1))
                        pp = sctx.enter_context(tc.tile_pool(name=f"sapt{l}", bufs=1))
                        kT_t = sp.tile([P, KC, S], bf16, tag="kT", name="kT_sa")
                        vaug_t = sp.tile([P, NKT, H * AVW], bf16, tag="vaug", name="vaug_sa")
                        qT_t = sp.tile([P, KC, HALF], bf16, tag="qT", name="qT_sa")
                        oT_t = sp.tile([P, KC, HALF], bf16, tag="oT", name="oT_sa")
                        u1 = lp.tile([P, KC, HALF], f32r, tag="u", bufs=1, name="u1")

                        oPart = sp.tile([AVW, 2 * H, TCH], bf16, tag="oPart",
                                        name="oPart")
                        kv_proj(sp, h_src,
                                saw["wk"], saw["bk"], saw["wv"], saw["bv"],
                                kT_t, vaug_t, ts_list=(0, 1))
                        q_proj(sp, hOwnT[:, :, :], saw["wq"], saw["bq"], qT_t)
                        attention_sa_own(pp, kT_t, vaug_t, qT_t, oPart)
                        # scheduling hint: peer h arrives only when the gather
                        # lands; without it the pass interleaves these matmuls
                        # into the own-chunk stream and the in-order PE queue
                        # stalls on them at runtime
                        with tc.tile_wait_until(WAIT_MS * l, enable=l > 0):
                            kv_proj(sp, h_src,
                                    saw["wk"], saw["bk"], saw["wv"], saw["bv"],
                                    kT_t, vaug_t, ts_list=(2, 3), memset_first=False)
                        attention_sa_peer(sp, pp, kT_t, vaug_t, qT_t, oT_t,
                                          oPart, identb_sb)
                        wo_sb = load_w(sp, w_in["sa_wo"], l, D, dt=bf16)
                        bo_sb = load_b(sp, w_in["sa_bo"], l, "bo")
                        out_proj(sp, oT_t, wo_sb, bo_sb,
                                 lambda ft, t: hOwnT[:, ft, t * TCH:(t + 1) * TCH], u1)
                        g1 = load_b(sp, w_in["ln1_g"], l, "g1") if ln_affine else None
                        b1l = load_b(sp, w_in["ln1_b"], l, "b1l") if ln_affine else None
                        layernorm(lp, rows, u1, lambda kc, t: x1T[:, kc, t * TCH:(t + 1) * TCH], g1, b1l)

                    # ---- cross-attention ----
                    cctx = contextlib.ExitStack()
                    with cctx:
                        cp = cctx.enter_context(tc.tile_pool(name=f"ca{l}", bufs=1))
                        pp2 = cctx.enter_context(tc.tile_pool(name=f"capt{l}", bufs=1))
                        kT_t = cp.tile([P, KC, S], bf16, tag="kT", name="kT_ca")
                        vaug_t = cp.tile([P, NKT, H * AVW], bf16, tag="vaug", name="vaug_ca")
                        qT_t = cp.tile([P, KC, HALF], bf16, tag="qT", name="qT_ca")
                        oT_t = cp.tile([P, KC, HALF], bf16, tag="oT", name="oT_ca")
                        u2 = lp.tile([P, KC, HALF], f32r, tag="u", bufs=1, name="u2")

                        def enc_chunk(t):
                            ec = cp.tile([P, KC, TCH], bf16, tag="enc", bufs=2, name="encC")
                            for kc in range(KC):
                                nc.sync.dma_start(out=ec[:, kc, :],
                                                  in_=encT[:, kc, t * TCH:(t + 1) * TCH])
                            return ec

                        wk_sb = load_w(cp, w_in["ca_wk"], l, D)
                        wv_sb = load_w(cp, w_in["ca_wv"], l, D)
                        bk_sb = load_b(cp, w_in["ca_bk"], l, "bk")
                        bv_sb = None
                        if v_bias:
                            bv_sb = cp.tile([1, D], f32, tag="bv", bufs=2, name="bv")
                            nc.sync.dma_start(out=bv_sb, in_=w_in["ca_bv"][l % L:l % L + 1, :])
                        kv_proj(cp, enc_chunk, wk_sb, bk_sb, wv_sb, bv_sb, kT_t, vaug_t)
                        wq_sb = load_w(cp, w_in["ca_wq"], l, D)
                        bq_sb = load_b(cp, w_in["ca_bq"], l, "bq")
                        q_proj(cp, x1T, wq_sb, bq_sb, qT_t)
                        attention_ca(cp, pp2, kT_t, vaug_t, qT_t, oT_t)
                        wo_sb = load_w(cp, w_in["ca_wo"], l, D, dt=bf16)
                        bo_sb = load_b(cp, w_in["ca_bo"], l, "bo")
                        out_proj(cp, oT_t, wo_sb, bo_sb,
                                 lambda ft, t: hOwnT[:, ft, t * TCH:(t + 1) * TCH], u2)
                        g2 = load_b(cp, w_in["ln2_g"], l, "g2") if ln_affine else None
                        b2l = load_b(cp, w_in["ln2_b"], l, "b2l") if ln_affine else None
                        layernorm(lp, rows, u2, lambda kc, t: yT[:, kc, t * TCH:(t + 1) * TCH], g2, b2l)

                    # ---- FFN ----
                    fctx = contextlib.ExitStack()
                    with fctx:
                        fp = fctx.enter_context(tc.tile_pool(name=f"ffn{l}", bufs=1))
                        w1_sb = load_w(fp, w_in["ff_w1"], l, F, dt=bf16, tag="w1", bufs=1)
                        b1_sb = load_b(fp, w_in["ff_b1"], l, "b1")
                        w2_sb = fp.tile([P, NFT, D], bf16, tag="w2", bufs=1, name="w2_sb")
                        for kc in range(NFT):
                            nc.sync.dma_start(out=w2_sb[:, kc, :],
                                              in_=w_in["ff_w2b"][l % L, kc * P:(kc + 1) * P, :])
                        b2_sb = load_b(fp, w_in["ff_b2"], l, "b2")
                        h1 = fp.tile([P, NFT, HALF], bf16, tag="h1", bufs=1, name="h1")
                        u3 = lp.tile([P, KC, HALF], f32r, tag="u", bufs=1, name="u3")
                        if l < NLAYERS - 1:
                            saw_next = load_sa_weights(l + 1)

                        for ft in range(NFT):
                            pss = [psA.tile([P, TCH], f32, tag="acc", name="f_ps") for _ in range(2)]
                            for kc in range(KC):
                                for t in range(2):
                                    nc.tensor.matmul(pss[t], w1_sb[:, kc, ft * P:(ft + 1) * P],
                                                     yT[:, kc, t * TCH:(t + 1) * TCH],
                                                     start=(kc == 0), stop=(kc == KC - 1))
                            for t in range(2):
                                nc.scalar.activation(h1[:, ft, t * TCH:(t + 1) * TCH],
                                                     pss[t], AF.Relu,
                                                     bias=b1_sb[:, ft:ft + 1])
                        for ft in range(KC):
                            pss = [psA.tile([P, TCH], f32, tag="acc", name="g_ps") for _ in range(2)]
                            for kc in range(NFT):
                                for t in range(2):
                                    nc.tensor.matmul(pss[t], w2_sb[:, kc, ft * P:(ft + 1) * P],
                                                     h1[:, kc, t * TCH:(t + 1) * TCH],
                                                     start=(kc == 0), stop=(kc == NFT - 1))
                            for t in range(2):
                                nc.vector.scalar_tensor_tensor(
                                    u3[:, ft, t * TCH:(t + 1) * TCH], pss[t],
                                    b2_sb[:, ft:ft + 1],
                                    x1T[:, ft, t * TCH:(t + 1) * TCH],
                                    ALU.add, ALU.add)
                        g3 = load_b(fp, w_in["ln3_g"], l, "g3") if ln_affine else None
                        b3l = load_b(fp, w_in["ln3_b"], l, "b3l") if ln_affine else None
                        if l == NLAYERS - 1:
                            dst3 = lambda kc, t: outT[:, kc, t * TCH:(t + 1) * TCH]
                        else:
                            dst3 = lambda kc, t: hOwnT[:, kc, t * TCH:(t + 1) * TCH]
                        layernorm(lp, rows, u3, dst3, g3, b3l)

                    # ---- single bf16 AllGather of the layer output (own half)
                    # within the pair ----
                    if l < NLAYERS - 1:
                        ccin = dramp.tile([P, KC, HALF], bf16, tag="ccin", bufs=2, name="ccin")
                        ccout = dramp.tile([2 * P, KC, HALF], bf16, tag="ccout", bufs=2,
                                           name="ccout")
                        nc.sync.dma_start(out=ccin, in_=hOwnT[:, :, :])
                        nc.gpsimd.collective_compute(
                            "AllGather", ALU.bypass, replica_groups=RG,
                            ins=[ccin.opt()], outs=[ccout.opt()])
                        with tc.tile_wait_until(WAIT_MS * (l + 1)):
                            nc.sync.dma_start(out=hPeerT[:, :, :],
                                              in_=ccout[ts(peer, P), :, :])
                        saw = saw_next

            nc.sync.dma_start(out=out_p[:, :, :], in_=outT[:, :, :].bitcast(f32))

    nc.finalize()
    return nc


_BUILD_CACHE = {}
LAST_RESULTS = None


def _get_nc(ln_affine, v_bias):
    key = (ln_affine, v_bias, NLAYERS, OPT_ACTSET, OPT_GMASK)
    if key not in _BUILD_CACHE:
        _BUILD_CACHE[key] = build(ln_affine, v_bias)
    return _BUILD_CACHE[key]


def _to_T(a):  # [S, D] -> [P, KC, S] feature-major
    return np.ascontiguousarray(a.T.reshape(KC, P, S).transpose(1, 0, 2))


def prepare(inputs):
    """Returns (nc, in_maps) for the given full inputs."""
    inp = {k: np.asarray(v) for k, v in inputs.items()}

    ln_affine = not all(
        np.all(inp[f"ln{i}_g"] == 1.0) and np.all(inp[f"ln{i}_b"] == 0.0) for i in (1, 2, 3)
    )
    v_bias = not (np.all(inp["sa_bv"] == 0.0) and np.all(inp["ca_bv"] == 0.0))
    nc = _get_nc(ln_affine, v_bias)

    ones = np.ones((P, P), np.float32)
    pcol = np.arange(P)[:, None]
    qcol = np.arange(TCH)[None, :]
    dmask = np.stack(
        [(qcol >= i * P + pcol) for i in range(4)], axis=1
    ).astype(ml_dtypes.bfloat16)  # [P, 4, TCH]

    shared = {}
    for pre in ("sa", "ca"):
        for nm in ("bq", "bk", "bv", "bo"):
            shared[f"{pre}_{nm}"] = np.ascontiguousarray(inp[f"{pre}_{nm}"], np.float32)
        for nm in ("wq", "wk", "wv", "wo"):
            shared[f"{pre}_{nm}"] = inp[f"{pre}_{nm}"].astype(ml_dtypes.bfloat16)
    shared["ff_w1"] = inp["ff_w1"].astype(ml_dtypes.bfloat16)
    shared["ff_b1"] = np.ascontiguousarray(inp["ff_b1"], np.float32)
    shared["ff_w2b"] = inp["ff_w2"].astype(ml_dtypes.bfloat16)
    shared["ff_b2"] = np.ascontiguousarray(inp["ff_b2"], np.float32)
    if ln_affine:
        for i in (1, 2, 3):
            shared[f"ln{i}_g"] = np.ascontiguousarray(inp[f"ln{i}_g"], np.float32)
            shared[f"ln{i}_b"] = np.ascontiguousarray(inp[f"ln{i}_b"], np.float32)
    shared["ones"] = ones
    shared["identb"] = np.eye(P, dtype=np.float32).astype(ml_dtypes.bfloat16)
    shared["dmask"] = dmask

    in_maps = []
    for r in range(8):
        b, g = r // 2, r % 2
        mine = [0, 3] if g == 0 else [1, 2]
        theirs = [1, 2] if g == 0 else [0, 3]
        perm = mine + theirs
        xt = np.concatenate([inp["x"][b].T[:, c * TCH:(c + 1) * TCH] for c in perm], axis=1)
        m = dict(shared)
        m["xT"] = np.ascontiguousarray(
            xt.reshape(KC, P, S).transpose(1, 0, 2)).astype(ml_dtypes.bfloat16)
        m["encT"] = _to_T(np.asarray(inp["enc"][b], np.float32)).astype(ml_dtypes.bfloat16)
        pb = np.zeros(12, np.float32)
        # exp-bias columns: j0 kt8-11 -> 0..3 ; j1 kt8-11 -> 4..7 ; j1 kt12-15 -> 8..11
        # Each group of 4 k-tiles lies in one peer global chunk kg; keep iff kg < qg.
        for base, j, kg in ((0, 0, theirs[0]), (4, 1, theirs[0]), (8, 1, theirs[1])):
            pb[base:base + 4] = 0.0 if kg < mine[j] else NEG
        m["pbias"] = np.broadcast_to(pb, (P, 12)).astype(np.float32).copy()
        in_maps.append(m)
    return nc, in_maps


def unshard(results):
    out = np.zeros((B, S, D), np.float32)
    for r in range(8):
        b, g = r // 2, r % 2
        mine = [0, 3] if g == 0 else [1, 2]
        half = results[r]["out"].transpose(1, 0, 2).reshape(D, HALF)
        for j, c in enumerate(mine):
            out[b, c * TCH:(c + 1) * TCH, :] = half[:, j * TCH:(j + 1) * TCH].T
    return out


def kernel(**inputs):
    global LAST_RESULTS
    nc, in_maps = prepare(inputs)

    res = None
    for attempt in range(3):
        try:
            res = run_bass_kernel_spmd(
                nc, in_maps, core_ids=list(range(8)),
                trace=bool(int(os.environ.get("KERNEL_TRACE", "0"))),
            )
            break
        except Exception:
            # first execution after a fresh NEFF compile occasionally flakes
            # on the runtime side; the NEFF cache makes the retry cheap
            if attempt == 2:
                raise
    LAST_RESULTS = res
    return unshard(res.results)

